# revision 27
# baseline (speedup 1.0000x reference)
"""Edge-parallel GNN message-passing kernel for 8 Trainium2 NeuronCores.

Strategy:
  * Host: sort edges by destination, split the edge list into 8 per-core
    shards at node boundaries (balanced edge counts); group each core's
    edges into <=512-edge groups whose destinations span <192 nodes.
  * The node-level src/dst linear transforms are folded into the first edge
    MLP layer on the host (they are linear, no activation in between).
  * Host pre-gathers the per-edge src/dst feature rows into transposed
    fp16 arrays [128, Gp*512]; the device streams them with large DMAs.
  * Device, per PAIR of 512-edge groups: edge MLP with fp32 PSUM
    accumulation using 1024-wide moving operands; b3 added via a rank-1
    ones-matmul on the PE.  LayerNorm stats via per-subtile bn_stats on
    DVE, combined chunk-wide (even/odd halves) with 6 strided DVE ops
    instead of per-subtile bn_aggr.  Mean-centering is fused into the
    PSUM->SBUF fp16 cast, alternating Scalar engine (mean-x) / DVE
    (x-mean); the sign difference is absorbed into per-subtile one-hot
    scales (+/- gate*rstd).
  * All Scalar-engine functions (Gelu/Identity/Tanh/Copy) live in one LUT
    set -> no ACT_TABLE_LOAD switches in steady state.  rstd comes from a
    Quake-style inverse-sqrt seed (DVE integer ops) plus two Newton steps
    on the otherwise-idle GPSIMD engine.
  * Gated, normalized messages are segment-summed into a 192-node window
    via an all-fp16 one-hot matmul, W_out applied, staging tiles written.
  * Host: accumulate the (overlapping) staging windows into the full
    [N, DOUT] output and add b_out.
"""

import os
import sys

sys.path.insert(0, "/opt/trn_rl_repo")

import numpy as np


def _ensure_ntff_hook():
    """The image's antenv package lacks axon_hooks; provide it so
    run_bass_kernel_spmd(trace=True) can capture NTFF profiles."""
    try:
        from antenv.axon_hooks import get_axon_ntff_profile_hook  # noqa: F401
        return
    except ImportError:
        pass
    import types
    try:
        from trn_agent_boot.trn_boot import _ntff_profile_via_ctypes
    except ImportError:
        return
    hook = _ntff_profile_via_ctypes("/opt/axon/libaxon_pjrt.so")
    if hook is None:
        return
    mod = types.ModuleType("antenv.axon_hooks")
    mod.get_axon_ntff_profile_hook = lambda: hook
    mod.set_axon_ntff_profile_hook = lambda h: None
    sys.modules["antenv.axon_hooks"] = mod
    try:
        import antenv
        antenv.axon_hooks = mod
    except ImportError:
        pass


_ensure_ntff_hook()

import concourse.bass as bass
import concourse.bacc as bacc
import concourse.tile as tile
from concourse import mybir
from concourse.bass_utils import run_bass_kernel_spmd

N_CORES = 8
H = 128
WINDOW = 192          # destination-node window per group
HW2 = WINDOW // 2     # window half for the W_out matmuls
GROUP_E = 512         # edges per group
SUBT = GROUP_E // 128  # 128-edge subtiles per group
PAIR_E = 2 * GROUP_E
CHUNK_G = 8           # groups per streamed chunk (must be even)
CHUNK_E = GROUP_E * CHUNK_G
NG = CHUNK_G * SUBT   # per-chunk (group, subtile) columns
LN_EPS = 1e-5
F16 = mybir.dt.float16
F32 = mybir.dt.float32
I32 = mybir.dt.int32
AF = mybir.ActivationFunctionType
ALU = mybir.AluOpType


# --------------------------------------------------------------------------
# host-side packing
# --------------------------------------------------------------------------

def _pack_core(ed_c):
    """Split one core's dst-sorted edges into groups of <=GROUP_E edges whose
    destinations span <WINDOW nodes.  Returns per-group (start, end, base)."""
    out = []
    i = 0
    n = len(ed_c)
    while i < n:
        base = int(ed_c[i])
        j_window = int(np.searchsorted(ed_c, base + WINDOW, side="left"))
        j = min(i + GROUP_E, j_window)
        out.append((i, j, base))
        i = j
    return out


def _prepare(inputs):
    feat = np.ascontiguousarray(np.asarray(inputs["feat"], np.float32))
    es = np.asarray(inputs["edge_src"]).astype(np.int64)
    ed = np.asarray(inputs["edge_dst"]).astype(np.int64)
    N, DIN = feat.shape
    E = es.shape[0]

    f64 = np.float64
    W_src = np.asarray(inputs["W_src"], f64)
    W_dst = np.asarray(inputs["W_dst"], f64)
    W1a = np.asarray(inputs["W1a"], f64)
    W1b = np.asarray(inputs["W1b"], f64)
    Wg1a = np.asarray(inputs["Wg1a"], f64)
    Wg1b = np.asarray(inputs["Wg1b"], f64)
    b_src = np.asarray(inputs["b_src"], f64)
    b_dst = np.asarray(inputs["b_dst"], f64)
    ln_g = np.asarray(inputs["ln_g"], f64)
    ln_b = np.asarray(inputs["ln_b"], f64)
    if not np.allclose(ln_b, 0.0):
        raise NotImplementedError("non-zero ln_b not supported")

    wpack = {
        "A1s": W_src @ W1a,
        "A1d": W_dst @ W1b,
        "Ag1s": W_src @ Wg1a,
        "Ag1d": W_dst @ Wg1b,
        "W2": np.asarray(inputs["W2"], f64),
        "W3": np.asarray(inputs["W3"], f64),
        "W_out": np.diag(ln_g) @ np.asarray(inputs["W_out"], f64),
    }
    b1f = np.asarray(inputs["b1"], f64) + b_src @ W1a + b_dst @ W1b
    bg1f = np.asarray(inputs["bg1"], f64) + b_src @ Wg1a + b_dst @ Wg1b
    Wg2 = np.asarray(inputs["Wg2"], f64)  # [H, 1]
    bg2 = float(np.asarray(inputs["bg2"], f64).reshape(()))

    feat16 = feat.astype(np.float16)

    order = np.argsort(ed, kind="stable")
    es_s = es[order]
    ed_s = ed[order]

    # split edges into 8 shards at node boundaries, balancing edge counts
    node_bounds = [0]
    for c in range(1, N_CORES):
        t = (c * E) // N_CORES
        node_bounds.append(int(ed_s[min(t, E - 1)]))
    node_bounds.append(N)
    bounds = np.searchsorted(ed_s, np.asarray(node_bounds), side="left")

    core_groups = []
    for c in range(N_CORES):
        lo, hi = int(bounds[c]), int(bounds[c + 1])
        core_groups.append(_pack_core(ed_s[lo:hi]))

    G = max(len(g) for g in core_groups)
    Gp = -(-G // CHUNK_G) * CHUNK_G
    nchunk = Gp // CHUNK_G

    in_maps = []
    meta = []
    for c in range(N_CORES):
        lo = int(bounds[c])
        groups = core_groups[c]
        src_idx = np.zeros((Gp, GROUP_E), np.int64)
        dst_idx = np.zeros((Gp, GROUP_E), np.int64)
        lidx = np.full((Gp, GROUP_E), -1.0, np.float32)
        bases = np.zeros(Gp, np.int64)
        for g, (i, j, base) in enumerate(groups):
            k = j - i
            src_idx[g, :k] = es_s[lo + i: lo + j]
            dst_idx[g, :k] = ed_s[lo + i: lo + j]
            lidx[g, :k] = (ed_s[lo + i: lo + j] - base).astype(np.float32)
            bases[g] = base

        fsrcT = np.ascontiguousarray(feat16[src_idx.ravel()].T)  # [128, Gp*GROUP_E]
        fdstT = np.ascontiguousarray(feat16[dst_idx.ravel()].T)

        # lidx transposed for per-partition access: [128, SUBT*Gp]
        lidxT = np.ascontiguousarray(
            lidx.reshape(Gp, SUBT, 128).transpose(2, 0, 1).reshape(128, -1)
        ).astype(np.float32)

        im = {
            "fsrcT": fsrcT,
            "fdstT": fdstT,
            "lidxT": lidxT,
            "b3rep": np.tile(np.asarray(inputs["b3"], np.float16), (1, 2 * SUBT)),
            "iota": np.tile(np.arange(WINDOW, dtype=np.float16), (128, 1)),
            "b1f": b1f.astype(np.float32).reshape(H, 1),
            "bg1f": bg1f.astype(np.float32).reshape(H, 1),
            "b2": np.asarray(inputs["b2"], np.float32).reshape(H, 1),
            "bg2": np.full((128, 1), 0.5 * bg2, np.float32),  # tanh-form sigmoid
            "Wg2": Wg2.astype(np.float16),
        }
        for k, v in wpack.items():
            im[k] = v.astype(np.float16)
        in_maps.append(im)
        meta.append(bases)

    b_out = np.asarray(inputs["b_out"], np.float64)
    return dict(in_maps=in_maps, meta=meta, Gp=Gp, nchunk=nchunk, N=N,
                b_out=b_out)


# --------------------------------------------------------------------------
# device kernel builder
# --------------------------------------------------------------------------

def _build(Gp, nchunk):
    nc = bacc.Bacc("TRN2", target_bir_lowering=False, debug=False)
    d = {}
    d["fsrcT"] = nc.dram_tensor("fsrcT", [128, Gp * GROUP_E], F16,
                                kind="ExternalInput")
    d["fdstT"] = nc.dram_tensor("fdstT", [128, Gp * GROUP_E], F16,
                                kind="ExternalInput")
    d["lidxT"] = nc.dram_tensor("lidxT", [128, SUBT * Gp], F32,
                                kind="ExternalInput")
    d["b3rep"] = nc.dram_tensor("b3rep", [1, 2 * SUBT * 128], F16,
                                kind="ExternalInput")
    d["iota"] = nc.dram_tensor("iota", [128, WINDOW], F16, kind="ExternalInput")
    for nm in ("b1f", "bg1f", "b2", "bg2"):
        d[nm] = nc.dram_tensor(nm, [128, 1], F32, kind="ExternalInput")
    for nm in ("A1s", "A1d", "Ag1s", "Ag1d", "W2", "W3", "W_out"):
        d[nm] = nc.dram_tensor(nm, [H, H], F16, kind="ExternalInput")
    d["Wg2"] = nc.dram_tensor("Wg2", [H, 1], F16, kind="ExternalInput")
    staging = nc.dram_tensor("staging", [Gp, 2, HW2, 128], F16,
                             kind="ExternalOutput")

    with tile.TileContext(nc) as tc:
        with (
            tc.tile_pool(name="singles", bufs=1) as singles,
            tc.tile_pool(name="gath", bufs=3) as gath,
            tc.tile_pool(name="acts", bufs=5) as acts,
            tc.tile_pool(name="msgs", bufs=12) as msgs,
            tc.tile_pool(name="ln", bufs=4) as lnp,
            tc.tile_pool(name="outp", bufs=5) as outp,
            tc.tile_pool(name="ppack", bufs=2, space="PSUM") as ppack,
            tc.tile_pool(name="psmall", bufs=1, space="PSUM") as psmall,
        ):
            # ---- preamble: constants into SBUF ----
            w = {}
            for nm in ("A1s", "A1d", "Ag1s", "Ag1d", "W2", "W3", "W_out"):
                w[nm] = singles.tile([H, H], F16, tag=nm, name=nm)
                nc.sync.dma_start(out=w[nm], in_=d[nm][:, :])
            w["Wg2"] = singles.tile([H, 1], F16, tag="Wg2", name="Wg2")
            nc.sync.dma_start(out=w["Wg2"], in_=d["Wg2"][:, :])
            bias = {}
            for nm in ("b1f", "bg1f", "b2", "bg2"):
                bias[nm] = singles.tile([128, 1], F32, tag=nm, name=nm)
                nc.sync.dma_start(out=bias[nm], in_=d[nm][:, :])
            b3rep = singles.tile([1, 2 * SUBT * 128], F16, tag="b3rep")
            nc.sync.dma_start(out=b3rep, in_=d["b3rep"][:, :])
            iota = singles.tile([128, WINDOW], F16, tag="iota")
            nc.sync.dma_start(out=iota, in_=d["iota"][:, :])
            lidxT = singles.tile([128, SUBT * Gp], F32, tag="lidxT")
            nc.sync.dma_start(out=lidxT, in_=d["lidxT"][:, :])
            ones16 = singles.tile([1, 128], F16, tag="ones16")
            nc.vector.memset(ones16, 1.0)
            # integer constants for the Quake-style rsqrt seed
            c_one = singles.tile([128, NG], I32, tag="c_one")
            nc.vector.memset(c_one, 1)
            c_neg1 = singles.tile([128, NG], I32, tag="c_neg1")
            nc.vector.memset(c_neg1, -1)
            c_magic = singles.tile([128, NG], I32, tag="c_magic")
            nc.vector.memset(c_magic, 0x5F3759E0)

            for c in range(nchunk):
                fsT = gath.tile([128, CHUNK_E], F16, tag="fsT")
                nc.sync.dma_start(
                    out=fsT, in_=d["fsrcT"][:, c * CHUNK_E:(c + 1) * CHUNK_E])
                fdT = gath.tile([128, CHUNK_E], F16, tag="fdT")
                nc.sync.dma_start(
                    out=fdT, in_=d["fdstT"][:, c * CHUNK_E:(c + 1) * CHUNK_E])

                # -- phase A (per pair of groups): edge MLP with 1024-wide
                #    moving operands; LN stats per subtile; centering fused
                #    into the PSUM->SBUF cast, alternating ACT/DVE --
                gatep = psmall.tile([128, NG], F32, tag="gatep", bufs=1)
                st_c = lnp.tile([128, NG, 6], F32, tag="st_c")
                mean_c = lnp.tile([128, NG], F32, tag="mean_c")
                msg16s = []
                for pi in range(CHUNK_G // 2):
                    e0 = pi * PAIR_E
                    fs = fsT[:, e0:e0 + PAIR_E]
                    fd = fdT[:, e0:e0 + PAIR_E]

                    HB = [slice(0, GROUP_E), slice(GROUP_E, PAIR_E)]
                    h1p = ppack.tile([128, PAIR_E], F32, tag="pair")
                    for hb in HB:
                        nc.tensor.matmul(h1p[:, hb], w["A1s"], fs[:, hb],
                                         start=True, stop=False,
                                         skip_group_check=True)
                    for hb in HB:
                        nc.tensor.matmul(h1p[:, hb], w["A1d"], fd[:, hb],
                                         start=False, stop=True,
                                         skip_group_check=True)
                    g1p = ppack.tile([128, PAIR_E], F32, tag="pair")
                    for hb in HB:
                        nc.tensor.matmul(g1p[:, hb], w["Ag1s"], fs[:, hb],
                                         start=True, stop=False,
                                         skip_group_check=True)
                    for hb in HB:
                        nc.tensor.matmul(g1p[:, hb], w["Ag1d"], fd[:, hb],
                                         start=False, stop=True,
                                         skip_group_check=True)

                    h1s = acts.tile([128, PAIR_E], F16, tag="h1s")
                    nc.scalar.activation(h1s, h1p, AF.Gelu, bias=bias["b1f"])
                    h2p = ppack.tile([128, PAIR_E], F32, tag="pair")
                    for hb in HB:
                        nc.tensor.matmul(h2p[:, hb], w["W2"], h1s[:, hb],
                                         start=True, stop=True,
                                         skip_group_check=True)
                    h2s = acts.tile([128, PAIR_E], F16, tag="h2s")
                    nc.scalar.activation(h2s, h2p, AF.Gelu, bias=bias["b2"])
                    g1s = acts.tile([128, PAIR_E], F16, tag="g1s")
                    nc.scalar.activation(g1s, g1p, AF.Gelu, bias=bias["bg1f"])

                    # msg_pre (un-transposed, [edge, feat]) and gate pre-act
                    msgp = ppack.tile([128, PAIR_E], F32, tag="pair")
                    for s in range(2 * SUBT):
                        sl = slice(s * 128, (s + 1) * 128)
                        nc.tensor.matmul(
                            msgp[:, sl], h2s[:, sl], w["W3"],
                            start=(s % SUBT == 0), stop=False,
                            skip_group_check=True,
                        )
                        k = pi * 2 * SUBT + s
                        nc.tensor.matmul(
                            gatep[:, k:k + 1], g1s[:, sl], w["Wg2"],
                            start=True, stop=True, skip_group_check=True,
                        )
                    # += b3 (rank-1 ones x b3rep), closing the accum groups
                    for hi, hb in enumerate(HB):
                        nc.tensor.matmul(
                            msgp[:, hb], ones16,
                            b3rep[:, hi * GROUP_E:(hi + 1) * GROUP_E],
                            start=False, stop=True, skip_group_check=True)

                    # per-subtile LN stats into the chunk-wide stats tile
                    msg16 = msgs.tile([128, PAIR_E], F16, tag="msg16")
                    k0 = pi * 2 * SUBT
                    ks = slice(k0, k0 + 2 * SUBT)
                    for s in range(2 * SUBT):
                        sl = slice(s * 128, (s + 1) * 128)
                        nc.vector.bn_stats(st_c[:, k0 + s, :], msgp[:, sl])
                    # pair-wise mean combine: mean = (me + mo)/2
                    nc.vector.tensor_tensor(
                        mean_c[:, ks], st_c[:, ks, 1], st_c[:, ks, 4],
                        op=ALU.add)
                    nc.vector.tensor_scalar(
                        mean_c[:, ks], mean_c[:, ks], 0.5, None, op0=ALU.mult)
                    # centering + fp16 cast (ACT: mean-x / DVE: x-mean)
                    for s in range(2 * SUBT):
                        sl = slice(s * 128, (s + 1) * 128)
                        k = k0 + s
                        if s % 2 == 0:
                            nc.scalar.activation(
                                msg16[:, sl], msgp[:, sl], AF.Identity,
                                bias=mean_c[:, k:k + 1], scale=-1.0,
                            )
                        else:
                            nc.vector.tensor_scalar(
                                msg16[:, sl], msgp[:, sl],
                                mean_c[:, k:k + 1], None, op0=ALU.subtract,
                            )
                    msg16s.append(msg16)

                # -- var combine (chunk-wide, strided views of st_c):
                #    var = (M2e+M2o)/128 + 0.25*(me-mo)^2
                dd = lnp.tile([128, NG], F32, tag="dd")
                nc.vector.tensor_tensor(dd, st_c[:, :, 1], st_c[:, :, 4],
                                        op=ALU.subtract)
                tq = lnp.tile([128, NG], F32, tag="tq")
                nc.vector.scalar_tensor_tensor(tq, dd, 0.25, dd,
                                               op0=ALU.mult, op1=ALU.mult)
                wt = lnp.tile([128, NG], F32, tag="wt")
                nc.vector.tensor_tensor(wt, st_c[:, :, 2], st_c[:, :, 5],
                                        op=ALU.add)
                nc.vector.scalar_tensor_tensor(wt, wt, 1.0 / 128.0, tq,
                                               op0=ALU.mult, op1=ALU.add)

                # -- phase B (chunk-wide): gate = (1+tanh)/2 via ACT tanh,
                #    rstd via Quake seed (DVE) + 2 Newton steps (GPSIMD) --
                gate_t = lnp.tile([128, NG], F32, tag="gate_t")
                nc.scalar.activation(gate_t, gatep, AF.Tanh,
                                     bias=bias["bg2"], scale=0.5)
                g2t = lnp.tile([128, NG], F32, tag="g2t")
                nc.gpsimd.tensor_scalar(g2t, gate_t, 1.0, None, op0=ALU.add)
                wte = lnp.tile([128, NG], F32, tag="wte")
                nc.gpsimd.tensor_scalar(wte, wt, LN_EPS, None, op0=ALU.add)
                ya = lnp.tile([128, NG], F32, tag="ya")
                yb = lnp.tile([128, NG], F32, tag="yb")
                tmp = lnp.tile([128, NG], F32, tag="tmp")
                ya_i, yb_i = ya.bitcast(I32), yb.bitcast(I32)
                nc.vector.tensor_tensor(ya_i, wte.bitcast(I32), c_one,
                                        op=ALU.logical_shift_right)
                nc.vector.tensor_tensor(yb_i, ya_i, c_neg1, op=ALU.bitwise_xor)
                nc.vector.tensor_tensor(ya_i, yb_i, c_magic, op=ALU.add)
                # Newton iter 1: ya = ya*(1.5 - 0.5*wte*ya^2)
                nc.gpsimd.tensor_tensor(tmp, ya, ya, op=ALU.mult)
                nc.gpsimd.tensor_tensor(tmp, tmp, wte, op=ALU.mult)
                nc.gpsimd.tensor_scalar(tmp, tmp, -0.5, 1.5,
                                        op0=ALU.mult, op1=ALU.add)
                nc.gpsimd.tensor_tensor(yb, ya, tmp, op=ALU.mult)
                # Newton iter 2 with 0.5 folded: ya = yb*(0.75 - 0.25*wte*yb^2)
                nc.gpsimd.tensor_tensor(tmp, yb, yb, op=ALU.mult)
                nc.gpsimd.tensor_tensor(tmp, tmp, wte, op=ALU.mult)
                nc.gpsimd.tensor_scalar(tmp, tmp, -0.25, 0.75,
                                        op0=ALU.mult, op1=ALU.add)
                nc.gpsimd.tensor_tensor(ya, yb, tmp, op=ALU.mult)
                sc_pos = lnp.tile([128, NG], F32, tag="sc_pos")
                nc.gpsimd.tensor_tensor(sc_pos, ya, g2t, op=ALU.mult)
                sc_neg = lnp.tile([128, NG], F32, tag="sc_neg")
                nc.gpsimd.tensor_scalar(sc_neg, sc_pos, -1.0, None,
                                        op0=ALU.mult)

                # -- phase C: one-hot (scaled, all fp16), segment-sum, W_out --
                for gi in range(CHUNK_G):
                    g = c * CHUNK_G + gi
                    pi, half = divmod(gi, 2)
                    msg16 = msg16s[pi]
                    eoff = half * GROUP_E
                    A = acts.tile([128, SUBT, WINDOW], F16, tag="A")
                    for s in range(SUBT):
                        k = gi * SUBT + s
                        sc = sc_neg if s % 2 == 0 else sc_pos
                        nc.vector.tensor_scalar(
                            A[:, s, :], iota,
                            lidxT[:, g * SUBT + s: g * SUBT + s + 1],
                            sc[:, k:k + 1],
                            op0=ALU.is_equal, op1=ALU.mult,
                        )

                    updp = psmall.tile([128, WINDOW], F32, tag="sm", bufs=2)
                    for s in range(SUBT):
                        sl = slice(eoff + s * 128, eoff + (s + 1) * 128)
                        nc.tensor.matmul(
                            updp, msg16[:, sl], A[:, s, :],
                            start=(s == 0), stop=(s == SUBT - 1),
                            skip_group_check=True,
                        )
                    upd16 = outp.tile([128, WINDOW], F16, tag="upd16")
                    if gi % 2 == 0:
                        nc.vector.tensor_copy(upd16, updp)
                    else:
                        nc.scalar.activation(upd16, updp, AF.Copy)

                    o2 = psmall.tile([HW2, 2, 128], F32, tag="o2", bufs=1)
                    for hh in range(2):
                        nc.tensor.matmul(
                            o2[:, hh, :], upd16[:, hh * HW2:(hh + 1) * HW2],
                            w["W_out"], start=True, stop=True,
                            skip_group_check=True,
                        )
                    osb = outp.tile([HW2, 2, 128], F16, tag="osb")
                    if gi % 2 == 0:
                        nc.scalar.activation(osb, o2, AF.Copy)
                    else:
                        nc.vector.tensor_copy(osb, o2)
                    nc.sync.dma_start(
                        out=staging[g].rearrange("hh j d -> j hh d"),
                        in_=osb,
                    )
    nc.finalize()
    return nc


# --------------------------------------------------------------------------
# entry point
# --------------------------------------------------------------------------

_LAST_PERF = {}


def kernel(**inputs):
    prep = _prepare(inputs)
    nc = _build(prep["Gp"], prep["nchunk"])
    trace = bool(int(os.environ.get("KERNEL_TRACE", "1")))
    res = run_bass_kernel_spmd(
        nc, prep["in_maps"], core_ids=list(range(N_CORES)), trace=trace,
    )
    _LAST_PERF.clear()
    _LAST_PERF.update(
        exec_time_ns=res.exec_time_ns,
        mean_exec_time_ns=res.mean_exec_time_ns,
        trace=res.instructions_and_trace[1] if res.instructions_and_trace else None,
        profile_json=res.profile_json,
    )

    N = prep["N"]
    out = np.zeros((N + WINDOW, H), np.float64)
    for c in range(N_CORES):
        stg = res.results[c]["staging"].reshape(prep["Gp"], WINDOW, H)
        bases = prep["meta"][c]
        for g in range(prep["Gp"]):
            b = int(bases[g])
            out[b: b + WINDOW] += stg[g]
    out = out[:N] + prep["b_out"]
    return out.astype(np.float32)
